# revision 26
# baseline (speedup 1.0000x reference)
"""Bahdanau attention (context + alpha) on Trainium2, 8-core data-parallel.

Math (per batch b):
  att1[p,a]  = sum_e enc[b,p,e] * W_enc[e,a]
  att2[a]    = sum_d dec[b,d] * W_dec[d,a] + b_dec[a]
  z[p,a]     = relu(att1[p,a] + att2[a] + b_enc[a])
  att[p]     = sum_a z[p,a] * w_full[a]          (+ b_full, dropped: softmax-shift-invariant)
  alpha[p]   = softmax_p(att)
  context[e] = sum_p alpha[p] * enc[b,p,e]

Sharding: batch dim split over 8 cores (32 batches each); small weights replicated.

On-chip layout: the att1 matmul contracts over e, so encoder tiles are needed with
e on the partition axis (encT).  Natural-layout tiles (p on partitions) are loaded
with fully contiguous DMA and transposed on the PE (identity matmul).  The att1
result is produced transposed (a on partitions, pixels of a batch PAIR side by
side on the free axis: N=392) so relu-bias (per-a) is a per-partition activation
bias and the w_full projection is one K=128 matmul per a-chunk.  Softmax runs on
the (1, 392) score row without max-subtraction (scores are O(sigma)~1, exp-safe).
Context reuses the natural-layout tiles: lhsT = transposed exp-row, accumulate
over the two p-chunks, scaled by 1/sum(exp) on PSUM->SBUF copy-out.
"""

import os
from contextlib import ExitStack

import numpy as np

import concourse.bass as bass
import concourse.mybir as mybir
import concourse.tile as tile
from concourse.bass_utils import run_bass_kernel_spmd
from concourse.masks import make_identity

F32 = mybir.dt.float32
AF = mybir.ActivationFunctionType

N_CORES = 8
B, P, E, A, D = 256, 196, 2048, 512, 512
BC = B // N_CORES            # 32 batches per core
NPAIR = BC // 2              # 16 batch pairs per core
P0 = 128                     # first p-chunk rows
P1 = P - P0                  # 68
ECH = E // 128               # 16 e-chunks
ACH = A // 128               # 4 a-chunks
DCH = D // 128               # 4 d-chunks
W2 = 2 * P                   # 392: paired free width

# Matmul dtype for the att1/score matmuls: float32 (exact, 4 cyc/row) or
# float32r (reduced-precision single-pass, 1 cyc/row at N>=256).  fp32r
# operands must be produced by ops that round to fp32r (walrus birverifier
# rule), so operand tiles are allocated in MM_DT and filled by compute ops,
# never straight from DMA.  The context matmul keeps plain fp32: its rhs is
# the DMA-loaded natural-layout encoder tile.
MM_DT = getattr(mybir.dt, os.environ.get("KERNEL_MM_DT", "float32r"))


def build(split_waits=True):
    nc = bass.Bass(
        trn_type="TRN2",
        target_bir_lowering=False,
        debug=False,
        num_devices=N_CORES,
    )

    enc_d = nc.dram_tensor("enc", [BC, P, E], F32, kind="ExternalInput").ap()
    dec_d = nc.dram_tensor("dec", [BC, D], F32, kind="ExternalInput").ap()
    wenc_d = nc.dram_tensor("w_enc", [E, A], F32, kind="ExternalInput").ap()
    benc_d = nc.dram_tensor("b_enc", [1, A], F32, kind="ExternalInput").ap()
    wdec_d = nc.dram_tensor("w_dec", [D, A], F32, kind="ExternalInput").ap()
    bdec_d = nc.dram_tensor("b_dec", [1, A], F32, kind="ExternalInput").ap()
    wful_d = nc.dram_tensor("w_full", [1, A], F32, kind="ExternalInput").ap()
    ctx_d = nc.dram_tensor("context", [BC, E], F32, kind="ExternalOutput").ap()
    alp_d = nc.dram_tensor("alpha", [BC, P], F32, kind="ExternalOutput").ap()

    with tile.TileContext(nc) as tc, ExitStack() as ctx:
        const = ctx.enter_context(tc.tile_pool(name="const", bufs=1))

        ident = const.tile([128, 128], F32)
        make_identity(nc, ident[:])

        # --- replicated weights ------------------------------------------
        wenc_r = const.tile([128, ECH * A], MM_DT)  # [ep, (ec, a)], rounded
        wful_t = const.tile([128, ACH], MM_DT)  # w_full as [a_in_chunk, ac]
        beb_t = const.tile([128, ACH], F32)     # b_enc+b_dec likewise
        att2p = const.tile([128, ACH * BC], F32)  # [ap, (ac, b)]: att2+biases

        with tc.tile_pool(name="setup_sb", bufs=1) as stage, \
                tc.tile_pool(name="setup_ps", bufs=2, space="PSUM") as sps:
            wenc_sb = stage.tile([128, ECH * A], F32)  # [ep, (ec, a)]
            for ec in range(ECH):
                nc.sync.dma_start(
                    wenc_sb[:, ec * A:(ec + 1) * A],
                    wenc_d[ec * 128:(ec + 1) * 128, :],
                )
                # round fp32 -> fp32r (per chunk: keeps DVE waits single-sem)
                nc.vector.tensor_copy(wenc_r[:, ec * A:(ec + 1) * A],
                                      wenc_sb[:, ec * A:(ec + 1) * A])
            wdec_sb = stage.tile([128, DCH * A], F32)  # [dp, (dc, a)]
            nc.sync.dma_start(
                wdec_sb[:].rearrange("p (c a) -> p c a", c=DCH),
                wdec_d.rearrange("(c p) a -> p c a", p=128),
            )
            benc_r = stage.tile([1, A], F32)
            nc.sync.dma_start(benc_r[:], benc_d)
            bdec_r = stage.tile([1, A], F32)
            nc.sync.dma_start(bdec_r[:], bdec_d)
            wful_r = stage.tile([1, A], F32)
            nc.sync.dma_start(wful_r[:], wful_d)
            dec_nat = stage.tile([BC, D], F32)
            nc.sync.dma_start(dec_nat[:], dec_d)

            # bias_eb = b_enc + b_dec (both added to att1 pre-relu).  Stage
            # bdec through a DVE copy so the add waits on one semaphore only
            # (DVE TensorTensor has a single sync-wait slot).
            bdec_c = stage.tile([1, A], F32)
            nc.vector.tensor_copy(bdec_c[:], bdec_r[:])
            beb_r = stage.tile([1, A], F32)
            nc.vector.tensor_add(beb_r[:], benc_r[:], bdec_c[:])
            dect_sb = stage.tile([128, DCH * BC], F32)  # decT: [dp, (dc, b)]

            for c in range(ACH):
                t = sps.tile([128, 1], F32, tag="vec")
                nc.tensor.transpose(t[:], wful_r[:, c * 128:(c + 1) * 128],
                                    ident[0:1, 0:1])
                nc.vector.tensor_copy(wful_t[:, c:c + 1], t[:])
                t2 = sps.tile([128, 1], F32, tag="vec")
                nc.tensor.transpose(t2[:], beb_r[:, c * 128:(c + 1) * 128],
                                    ident[0:1, 0:1])
                nc.vector.tensor_copy(beb_t[:, c:c + 1], t2[:])
            for dc in range(DCH):
                t = sps.tile([128, BC], F32, tag="dec")
                nc.tensor.transpose(t[:], dec_nat[:, dc * 128:(dc + 1) * 128],
                                    ident[0:BC, 0:BC])
                nc.vector.tensor_copy(dect_sb[:, dc * BC:(dc + 1) * BC], t[:])
            # att2p[:, ac*BC + b] = (dec @ W_dec)[b, ac*128:...] + b_enc + b_dec
            for ac in range(ACH):
                t = sps.tile([128, BC], F32, tag="att2")
                for dc in range(DCH):
                    nc.tensor.matmul(
                        t[:],
                        wdec_sb[:, dc * A + ac * 128: dc * A + (ac + 1) * 128],
                        dect_sb[:, dc * BC:(dc + 1) * BC],
                        start=(dc == 0),
                        stop=(dc == DCH - 1),
                    )
                nc.vector.tensor_scalar_add(att2p[:, ac * BC:(ac + 1) * BC],
                                            t[:], beb_t[:, ac:ac + 1])

        # --- main pools ---------------------------------------------------
        stgA = ctx.enter_context(tc.tile_pool(name="stgA", bufs=2))
        stgB = ctx.enter_context(tc.tile_pool(name="stgB", bufs=2))
        natA = ctx.enter_context(tc.tile_pool(name="natA", bufs=4))
        natB = ctx.enter_context(tc.tile_pool(name="natB", bufs=4))
        encT = ctx.enter_context(tc.tile_pool(name="encT", bufs=2))
        relu_p = ctx.enter_context(tc.tile_pool(name="relu", bufs=2))
        sm_p = ctx.enter_context(tc.tile_pool(name="sm", bufs=2))
        ctxrow_p = ctx.enter_context(tc.tile_pool(name="ctxrow", bufs=2))
        aT_sb_p = ctx.enter_context(tc.tile_pool(name="aTsb", bufs=2))

        eT_ps_p = ctx.enter_context(tc.tile_pool(name="eTps", bufs=2, space="PSUM"))
        z_ps_p = ctx.enter_context(tc.tile_pool(name="zps", bufs=2, space="PSUM"))
        att_ps_p = ctx.enter_context(tc.tile_pool(name="attps", bufs=1, space="PSUM"))
        aT_ps_p = ctx.enter_context(tc.tile_pool(name="aTps", bufs=1, space="PSUM"))
        ctx_ps_p = ctx.enter_context(tc.tile_pool(name="ctxps", bufs=2, space="PSUM"))

        for i in range(NPAIR):
            b0, b1 = 2 * i, 2 * i + 1

            # DMA lands fp32 in a short-lived staging tile; a round-copy to
            # MM_DT produces the tile every on-chip consumer reads (the
            # birverifier requires fp32r matmul operands to come from a
            # rounding instruction, and DMA cannot round).  Copies alternate
            # DVE/ACT to balance engine load.
            nat = []
            for j, b in enumerate((b0, b1)):
                sa = stgA.tile([P0, E], F32, tag="stgA")
                nc.sync.dma_start(sa[:], enc_d[b, 0:P0, :])
                a_t = natA.tile([P0, E], MM_DT, tag="natA")
                nc.vector.tensor_copy(a_t[:], sa[:])
                sb = stgB.tile([P1, E], F32, tag="stgB")
                nc.sync.dma_start(sb[:], enc_d[b, P0:P, :])
                b_t = natB.tile([P1, E], MM_DT, tag="natB")
                nc.vector.tensor_copy(b_t[:], sb[:])
                nat.append((a_t, b_t))

            # encT: [e_in_chunk, (ec, pair-cols)]; cols = b0 p0..p195, b1 p0..p195
            eT = encT.tile([128, ECH * W2], MM_DT, tag="encT")
            for ec in range(ECH):
                ps = eT_ps_p.tile([128, W2], F32, tag="eT")
                sl = ec * 128
                nc.tensor.transpose(ps[:, 0:P0],
                                    nat[0][0][:, sl:sl + 128].bitcast(F32),
                                    ident[:])
                nc.tensor.transpose(ps[:, P0:P],
                                    nat[0][1][:, sl:sl + 128].bitcast(F32),
                                    ident[0:P1, 0:P1])
                nc.tensor.transpose(ps[:, P:P + P0],
                                    nat[1][0][:, sl:sl + 128].bitcast(F32),
                                    ident[:])
                nc.tensor.transpose(ps[:, P + P0:W2],
                                    nat[1][1][:, sl:sl + 128].bitcast(F32),
                                    ident[0:P1, 0:P1])
                nc.vector.tensor_copy(eT[:, ec * W2:(ec + 1) * W2], ps[:])

            # att scores for the pair: (1, 392) accumulated over a-chunks
            att_ps = att_ps_p.tile([1, W2], F32, tag="att")
            for ac in range(ACH):
                z = z_ps_p.tile([128, W2], F32, tag="z")
                for ec in range(ECH):
                    nc.tensor.matmul(
                        z[:],
                        wenc_r[:, ec * A + ac * 128: ec * A + (ac + 1) * 128],
                        eT[:, ec * W2:(ec + 1) * W2],
                        start=(ec == 0),
                        stop=(ec == ECH - 1),
                    )
                # r = relu(z + att2[b]) on DVE, rounding to fp32r for score mm
                r = relu_p.tile([128, W2], MM_DT, tag="relu")
                nc.vector.tensor_scalar(
                    r[:, 0:P], z[:, 0:P],
                    att2p[:, ac * BC + b0: ac * BC + b0 + 1], 0.0,
                    op0=mybir.AluOpType.add, op1=mybir.AluOpType.max)
                nc.vector.tensor_scalar(
                    r[:, P:W2], z[:, P:W2],
                    att2p[:, ac * BC + b1: ac * BC + b1 + 1], 0.0,
                    op0=mybir.AluOpType.add, op1=mybir.AluOpType.max)
                nc.tensor.matmul(att_ps[:], wful_t[:, ac:ac + 1], r[:],
                                 start=(ac == 0), stop=(ac == ACH - 1))

            # softmax over each 196-half (no max-subtraction; scores are O(1))
            exp_sb = sm_p.tile([1, W2], F32, tag="exp")
            s_sb = sm_p.tile([1, 2], F32, tag="s")
            rec = sm_p.tile([1, 2], F32, tag="rec")
            nc.scalar.activation(exp_sb[:, 0:P], att_ps[:, 0:P], AF.Exp,
                                 accum_out=s_sb[:, 0:1])
            nc.scalar.activation(exp_sb[:, P:W2], att_ps[:, P:W2], AF.Exp,
                                 accum_out=s_sb[:, 1:2])
            nc.vector.reciprocal(rec[:], s_sb[:])
            alpha_sb = sm_p.tile([1, W2], F32, tag="alpha")
            nc.vector.tensor_scalar_mul(alpha_sb[:, 0:P], exp_sb[:, 0:P],
                                        rec[:, 0:1])
            nc.vector.tensor_scalar_mul(alpha_sb[:, P:W2], exp_sb[:, P:W2],
                                        rec[:, 1:2])
            nc.sync.dma_start(alp_d[b0:b0 + 1, :], alpha_sb[0:1, 0:P])
            nc.sync.dma_start(alp_d[b1:b1 + 1, :], alpha_sb[0:1, P:W2])

            # transpose normalized alpha row -> column vectors for context
            aT_ps = aT_ps_p.tile([128, 4], F32, tag="aT")
            nc.tensor.transpose(aT_ps[:, 0:1], alpha_sb[:, 0:P0], ident[0:1, 0:1])
            nc.tensor.transpose(aT_ps[:, 1:2], alpha_sb[:, P:P + P0],
                                ident[0:1, 0:1])
            nc.tensor.transpose(aT_ps[0:P1, 2:3], alpha_sb[:, P0:P],
                                ident[0:1, 0:1])
            nc.tensor.transpose(aT_ps[0:P1, 3:4], alpha_sb[:, P + P0:W2],
                                ident[0:1, 0:1])
            aT = aT_sb_p.tile([128, 4], MM_DT, tag="aTsb")
            nc.vector.tensor_copy(aT[:, 0:2], aT_ps[:, 0:2])
            nc.vector.tensor_copy(aT[0:P1, 2:4], aT_ps[0:P1, 2:4])

            # context[b] = (sum_p exp[p] * enc[p, :]) / sum_exp
            for j, b in enumerate((b0, b1)):
                crow = ctxrow_p.tile([1, E], F32, tag="ctxrow")
                for n4 in range(4):
                    cps = ctx_ps_p.tile([1, 512], F32, tag="ctx")
                    nc.tensor.matmul(cps[:], aT[:, j:j + 1],
                                     nat[j][0][:, n4 * 512:(n4 + 1) * 512],
                                     start=True, stop=False)
                    nc.tensor.matmul(cps[:], aT[0:P1, 2 + j:3 + j],
                                     nat[j][1][:, n4 * 512:(n4 + 1) * 512],
                                     start=False, stop=True)
                    nc.scalar.activation(crow[:, n4 * 512:(n4 + 1) * 512], cps[:],
                                         AF.Copy)
                nc.sync.dma_start(ctx_d[b:b + 1, :], crow[:])

    # CoreSim can't model the raw inserted wait ops; skip the split there.
    return _split_multi_waits(nc) if split_waits else nc


# Instruction classes whose waits live outside the 64B engine encoding.
_WAIT_SPLIT_SKIP = {"InstEventSemaphore", "InstCollectiveCompute"}


def _split_multi_waits(nc):
    """The 64-byte ISA encoding has exactly ONE semaphore-wait slot per
    instruction; this walrus build refuses instructions carrying more.  Tile's
    sem-assignment can attach several, so split the extras into standalone
    EventSemaphore (wait-only) instructions on the same engine, inserted
    immediately before the over-subscribed instruction."""
    wid = 0
    for f in nc.m.functions:
        for blk in f.blocks:
            il = blk.instructions
            i = 0
            while i < len(il):
                inst = il[i]
                si = getattr(inst, "sync_info", None)
                if (si is not None and len(si.on_wait) > 1
                        and type(inst).__name__ not in _WAIT_SPLIT_SKIP):
                    for w in si.on_wait[:-1]:
                        ws = mybir.InstEventSemaphore(name=f"I-wsplit-{wid}")
                        wid += 1
                        ws.engine = inst.engine
                        ws.sync_info = mybir.SyncInfo(on_wait=[w], on_update=[])
                        il.insert(i, ws)
                        i += 1
                    inst.sync_info = mybir.SyncInfo(on_wait=si.on_wait[-1:],
                                                    on_update=si.on_update)
                i += 1
    return nc


_CACHE = {}


def kernel(encoder_out, decoder_hidden, W_enc, b_enc, W_dec, b_dec, w_full,
           b_full=None, **_ignored):
    encoder_out = np.ascontiguousarray(encoder_out, dtype=np.float32)
    decoder_hidden = np.ascontiguousarray(decoder_hidden, dtype=np.float32)
    shared = {
        "w_enc": np.ascontiguousarray(W_enc, dtype=np.float32),
        "b_enc": np.ascontiguousarray(b_enc, dtype=np.float32).reshape(1, A),
        "w_dec": np.ascontiguousarray(W_dec, dtype=np.float32),
        "b_dec": np.ascontiguousarray(b_dec, dtype=np.float32).reshape(1, A),
        "w_full": np.ascontiguousarray(w_full, dtype=np.float32).reshape(1, A),
    }
    if "nc" not in _CACHE:
        _CACHE["nc"] = build()
    nc = _CACHE["nc"]

    in_maps = []
    for c in range(N_CORES):
        sl = slice(c * BC, (c + 1) * BC)
        in_maps.append({
            "enc": encoder_out[sl],
            "dec": decoder_hidden[sl],
            **shared,
        })
    res = run_bass_kernel_spmd(nc, in_maps, list(range(N_CORES)))
    context = np.concatenate([r["context"] for r in res.results], axis=0)
    alpha = np.concatenate([r["alpha"] for r in res.results], axis=0)
    return context, alpha


# revision 28
# speedup vs baseline: 1.3283x; 1.3283x over previous
"""Bahdanau attention (context + alpha) on Trainium2, 8-core data-parallel.

Math (per batch b):
  att1[p,a]  = sum_e enc[b,p,e] * W_enc[e,a]
  att2[a]    = sum_d dec[b,d] * W_dec[d,a] + b_dec[a]
  z[p,a]     = relu(att1[p,a] + att2[a] + b_enc[a])
  att[p]     = sum_a z[p,a] * w_full[a]          (+ b_full, dropped: softmax-shift-invariant)
  alpha[p]   = softmax_p(att)
  context[e] = sum_p alpha[p] * enc[b,p,e]

Sharding: batch dim split over 8 cores (32 batches each); small weights replicated.

On-chip layout: the att1 matmul contracts over e, so encoder tiles are needed with
e on the partition axis (encT).  Natural-layout tiles (p on partitions) are loaded
with fully contiguous DMA and transposed on the PE (identity matmul).  The att1
result is produced transposed (a on partitions, pixels of a batch PAIR side by
side on the free axis: N=392) so relu-bias (per-a) is a per-partition activation
bias and the w_full projection is one K=128 matmul per a-chunk.  Softmax runs on
the (1, 392) score row without max-subtraction (scores are O(sigma)~1, exp-safe).
Context reuses the natural-layout tiles: lhsT = transposed exp-row, accumulate
over the two p-chunks, scaled by 1/sum(exp) on PSUM->SBUF copy-out.
"""

import os
from contextlib import ExitStack

import numpy as np

import concourse.bass as bass
import concourse.mybir as mybir
import concourse.tile as tile
from concourse.bass_utils import run_bass_kernel_spmd
from concourse.masks import make_identity

F32 = mybir.dt.float32
AF = mybir.ActivationFunctionType

N_CORES = 8
B, P, E, A, D = 256, 196, 2048, 512, 512
BC = B // N_CORES            # 32 batches per core
NPAIR = BC // 2              # 16 batch pairs per core
P0 = 128                     # first p-chunk rows
P1 = P - P0                  # 68
ECH = E // 128               # 16 e-chunks
ACH = A // 128               # 4 a-chunks
DCH = D // 128               # 4 d-chunks
W2 = 2 * P                   # 392: paired free width

# Matmul dtype for the att1/score matmuls: float32 (exact, 4 cyc/row) or
# float32r (reduced-precision single-pass, 1 cyc/row at N>=256).  fp32r
# operands must be produced by ops that round to fp32r (walrus birverifier
# rule), so operand tiles are allocated in MM_DT and filled by compute ops,
# never straight from DMA.  The context matmul keeps plain fp32: its rhs is
# the DMA-loaded natural-layout encoder tile.
MM_DT = getattr(mybir.dt, os.environ.get("KERNEL_MM_DT", "float32r"))


def build(split_waits=True):
    nc = bass.Bass(
        trn_type="TRN2",
        target_bir_lowering=False,
        debug=False,
        num_devices=N_CORES,
    )

    enc_d = nc.dram_tensor("enc", [BC, P, E], F32, kind="ExternalInput").ap()
    dec_d = nc.dram_tensor("dec", [BC, D], F32, kind="ExternalInput").ap()
    wenc_d = nc.dram_tensor("w_enc", [E, A], F32, kind="ExternalInput").ap()
    benc_d = nc.dram_tensor("b_enc", [1, A], F32, kind="ExternalInput").ap()
    wdec_d = nc.dram_tensor("w_dec", [D, A], F32, kind="ExternalInput").ap()
    bdec_d = nc.dram_tensor("b_dec", [1, A], F32, kind="ExternalInput").ap()
    wful_d = nc.dram_tensor("w_full", [1, A], F32, kind="ExternalInput").ap()
    ctx_d = nc.dram_tensor("context", [BC, E], F32, kind="ExternalOutput").ap()
    alp_d = nc.dram_tensor("alpha", [BC, P], F32, kind="ExternalOutput").ap()

    with tile.TileContext(nc) as tc, ExitStack() as ctx:
        const = ctx.enter_context(tc.tile_pool(name="const", bufs=1))

        ident = const.tile([128, 128], F32)
        make_identity(nc, ident[:])
        # rounded identity: fp32r transposes run 1.5 cyc/row vs 2.0 for fp32
        ident_r = const.tile([128, 128], MM_DT)
        nc.vector.tensor_copy(ident_r[:], ident[:])

        # --- replicated weights ------------------------------------------
        wenc_r = const.tile([128, ECH * A], MM_DT)  # [ep, (ec, a)], rounded
        wful_t = const.tile([128, ACH], MM_DT)  # w_full as [a_in_chunk, ac]
        beb_t = const.tile([128, ACH], F32)     # b_enc+b_dec likewise
        att2p = const.tile([128, ACH * BC], F32)  # [ap, (ac, b)]: att2+biases

        with tc.tile_pool(name="setup_sb", bufs=1) as stage, \
                tc.tile_pool(name="setup_ps", bufs=2, space="PSUM") as sps:
            wenc_sb = stage.tile([128, ECH * A], F32)  # [ep, (ec, a)]
            for ec in range(ECH):
                nc.sync.dma_start(
                    wenc_sb[:, ec * A:(ec + 1) * A],
                    wenc_d[ec * 128:(ec + 1) * 128, :],
                )
                # round fp32 -> fp32r (per chunk: keeps DVE waits single-sem)
                nc.vector.tensor_copy(wenc_r[:, ec * A:(ec + 1) * A],
                                      wenc_sb[:, ec * A:(ec + 1) * A])
            wdec_sb = stage.tile([128, DCH * A], F32)  # [dp, (dc, a)]
            nc.sync.dma_start(
                wdec_sb[:].rearrange("p (c a) -> p c a", c=DCH),
                wdec_d.rearrange("(c p) a -> p c a", p=128),
            )
            benc_r = stage.tile([1, A], F32)
            nc.sync.dma_start(benc_r[:], benc_d)
            bdec_r = stage.tile([1, A], F32)
            nc.sync.dma_start(bdec_r[:], bdec_d)
            wful_r = stage.tile([1, A], F32)
            nc.sync.dma_start(wful_r[:], wful_d)
            dec_nat = stage.tile([BC, D], F32)
            nc.sync.dma_start(dec_nat[:], dec_d)

            # bias_eb = b_enc + b_dec (both added to att1 pre-relu).  Stage
            # bdec through a DVE copy so the add waits on one semaphore only
            # (DVE TensorTensor has a single sync-wait slot).
            bdec_c = stage.tile([1, A], F32)
            nc.vector.tensor_copy(bdec_c[:], bdec_r[:])
            beb_r = stage.tile([1, A], F32)
            nc.vector.tensor_add(beb_r[:], benc_r[:], bdec_c[:])
            dect_sb = stage.tile([128, DCH * BC], F32)  # decT: [dp, (dc, b)]

            for c in range(ACH):
                t = sps.tile([128, 1], F32, tag="vec")
                nc.tensor.transpose(t[:], wful_r[:, c * 128:(c + 1) * 128],
                                    ident[0:1, 0:1])
                nc.vector.tensor_copy(wful_t[:, c:c + 1], t[:])
                t2 = sps.tile([128, 1], F32, tag="vec")
                nc.tensor.transpose(t2[:], beb_r[:, c * 128:(c + 1) * 128],
                                    ident[0:1, 0:1])
                nc.vector.tensor_copy(beb_t[:, c:c + 1], t2[:])
            for dc in range(DCH):
                t = sps.tile([128, BC], F32, tag="dec")
                nc.tensor.transpose(t[:], dec_nat[:, dc * 128:(dc + 1) * 128],
                                    ident[0:BC, 0:BC])
                nc.vector.tensor_copy(dect_sb[:, dc * BC:(dc + 1) * BC], t[:])
            # att2p[:, ac*BC + b] = (dec @ W_dec)[b, ac*128:...] + b_enc + b_dec
            for ac in range(ACH):
                t = sps.tile([128, BC], F32, tag="att2")
                for dc in range(DCH):
                    nc.tensor.matmul(
                        t[:],
                        wdec_sb[:, dc * A + ac * 128: dc * A + (ac + 1) * 128],
                        dect_sb[:, dc * BC:(dc + 1) * BC],
                        start=(dc == 0),
                        stop=(dc == DCH - 1),
                    )
                nc.vector.tensor_scalar_add(att2p[:, ac * BC:(ac + 1) * BC],
                                            t[:], beb_t[:, ac:ac + 1])

        # --- main pools ---------------------------------------------------
        stgA = ctx.enter_context(tc.tile_pool(name="stgA", bufs=2))
        stgB = ctx.enter_context(tc.tile_pool(name="stgB", bufs=2))
        natA = ctx.enter_context(tc.tile_pool(name="natA", bufs=4))
        natB = ctx.enter_context(tc.tile_pool(name="natB", bufs=4))
        encT = ctx.enter_context(tc.tile_pool(name="encT", bufs=2))
        relu_p = ctx.enter_context(tc.tile_pool(name="relu", bufs=2))
        sm_p = ctx.enter_context(tc.tile_pool(name="sm", bufs=2))
        ctxrow_p = ctx.enter_context(tc.tile_pool(name="ctxrow", bufs=2))
        aT_sb_p = ctx.enter_context(tc.tile_pool(name="aTsb", bufs=2))

        eT_ps_p = ctx.enter_context(tc.tile_pool(name="eTps", bufs=2, space="PSUM"))
        z_ps_p = ctx.enter_context(tc.tile_pool(name="zps", bufs=2, space="PSUM"))
        att_ps_p = ctx.enter_context(tc.tile_pool(name="attps", bufs=1, space="PSUM"))
        aT_ps_p = ctx.enter_context(tc.tile_pool(name="aTps", bufs=1, space="PSUM"))
        ctx_ps_p = ctx.enter_context(tc.tile_pool(name="ctxps", bufs=2, space="PSUM"))

        for i in range(NPAIR):
            b0, b1 = 2 * i, 2 * i + 1

            # DMA lands fp32 in a short-lived staging tile; a round-copy to
            # MM_DT produces the tile every on-chip consumer reads (the
            # birverifier requires fp32r matmul operands to come from a
            # rounding instruction, and DMA cannot round).  Copies alternate
            # DVE/ACT to balance engine load.
            nat = []
            for j, b in enumerate((b0, b1)):
                sa = stgA.tile([P0, E], F32, tag="stgA")
                nc.sync.dma_start(sa[:], enc_d[b, 0:P0, :])
                a_t = natA.tile([P0, E], MM_DT, tag="natA")
                nc.vector.tensor_copy(a_t[:], sa[:])
                sb = stgB.tile([P1, E], F32, tag="stgB")
                nc.sync.dma_start(sb[:], enc_d[b, P0:P, :])
                b_t = natB.tile([P1, E], MM_DT, tag="natB")
                nc.vector.tensor_copy(b_t[:], sb[:])
                nat.append((a_t, b_t))

            # encT: [e_in_chunk, (ec, pair-cols)]; cols = b0 p0..p195, b1 p0..p195
            eT = encT.tile([128, ECH * W2], MM_DT, tag="encT")
            for ec in range(ECH):
                ps = eT_ps_p.tile([128, W2], MM_DT, tag="eT")
                sl = ec * 128
                nc.tensor.transpose(ps[:, 0:P0], nat[0][0][:, sl:sl + 128],
                                    ident_r[:])
                nc.tensor.transpose(ps[:, P0:P], nat[0][1][:, sl:sl + 128],
                                    ident_r[0:P1, 0:P1])
                nc.tensor.transpose(ps[:, P:P + P0], nat[1][0][:, sl:sl + 128],
                                    ident_r[:])
                nc.tensor.transpose(ps[:, P + P0:W2], nat[1][1][:, sl:sl + 128],
                                    ident_r[0:P1, 0:P1])
                nc.vector.tensor_copy(eT[:, ec * W2:(ec + 1) * W2], ps[:])

            # att scores for the pair: (1, 392) accumulated over a-chunks
            att_ps = att_ps_p.tile([1, W2], F32, tag="att")
            for ac in range(ACH):
                z = z_ps_p.tile([128, W2], F32, tag="z")
                for ec in range(ECH):
                    nc.tensor.matmul(
                        z[:],
                        wenc_r[:, ec * A + ac * 128: ec * A + (ac + 1) * 128],
                        eT[:, ec * W2:(ec + 1) * W2],
                        start=(ec == 0),
                        stop=(ec == ECH - 1),
                    )
                # r = relu(z + att2[b]) on DVE, rounding to fp32r for score mm
                r = relu_p.tile([128, W2], MM_DT, tag="relu")
                nc.vector.tensor_scalar(
                    r[:, 0:P], z[:, 0:P],
                    att2p[:, ac * BC + b0: ac * BC + b0 + 1], 0.0,
                    op0=mybir.AluOpType.add, op1=mybir.AluOpType.max)
                nc.vector.tensor_scalar(
                    r[:, P:W2], z[:, P:W2],
                    att2p[:, ac * BC + b1: ac * BC + b1 + 1], 0.0,
                    op0=mybir.AluOpType.add, op1=mybir.AluOpType.max)
                nc.tensor.matmul(att_ps[:], wful_t[:, ac:ac + 1], r[:],
                                 start=(ac == 0), stop=(ac == ACH - 1))

            # softmax over each 196-half (no max-subtraction; scores are O(1))
            exp_sb = sm_p.tile([1, W2], F32, tag="exp")
            s_sb = sm_p.tile([1, 2], F32, tag="s")
            rec = sm_p.tile([1, 2], F32, tag="rec")
            nc.scalar.activation(exp_sb[:, 0:P], att_ps[:, 0:P], AF.Exp,
                                 accum_out=s_sb[:, 0:1])
            nc.scalar.activation(exp_sb[:, P:W2], att_ps[:, P:W2], AF.Exp,
                                 accum_out=s_sb[:, 1:2])
            nc.vector.reciprocal(rec[:], s_sb[:])
            alpha_sb = sm_p.tile([1, W2], F32, tag="alpha")
            nc.vector.tensor_scalar_mul(alpha_sb[:, 0:P], exp_sb[:, 0:P],
                                        rec[:, 0:1])
            nc.vector.tensor_scalar_mul(alpha_sb[:, P:W2], exp_sb[:, P:W2],
                                        rec[:, 1:2])
            nc.sync.dma_start(alp_d[b0:b0 + 1, :], alpha_sb[0:1, 0:P])
            nc.sync.dma_start(alp_d[b1:b1 + 1, :], alpha_sb[0:1, P:W2])

            # transpose normalized alpha row -> column vectors for context
            aT_ps = aT_ps_p.tile([128, 4], F32, tag="aT")
            nc.tensor.transpose(aT_ps[:, 0:1], alpha_sb[:, 0:P0], ident[0:1, 0:1])
            nc.tensor.transpose(aT_ps[:, 1:2], alpha_sb[:, P:P + P0],
                                ident[0:1, 0:1])
            nc.tensor.transpose(aT_ps[0:P1, 2:3], alpha_sb[:, P0:P],
                                ident[0:1, 0:1])
            nc.tensor.transpose(aT_ps[0:P1, 3:4], alpha_sb[:, P + P0:W2],
                                ident[0:1, 0:1])
            aT = aT_sb_p.tile([128, 4], MM_DT, tag="aTsb")
            nc.vector.tensor_copy(aT[:, 0:2], aT_ps[:, 0:2])
            nc.vector.tensor_copy(aT[0:P1, 2:4], aT_ps[0:P1, 2:4])

            # context[b] = (sum_p exp[p] * enc[p, :]) / sum_exp
            for j, b in enumerate((b0, b1)):
                crow = ctxrow_p.tile([1, E], F32, tag="ctxrow")
                for n4 in range(4):
                    cps = ctx_ps_p.tile([1, 512], F32, tag="ctx")
                    nc.tensor.matmul(cps[:], aT[:, j:j + 1],
                                     nat[j][0][:, n4 * 512:(n4 + 1) * 512],
                                     start=True, stop=False)
                    nc.tensor.matmul(cps[:], aT[0:P1, 2 + j:3 + j],
                                     nat[j][1][:, n4 * 512:(n4 + 1) * 512],
                                     start=False, stop=True)
                    nc.scalar.activation(crow[:, n4 * 512:(n4 + 1) * 512], cps[:],
                                         AF.Copy)
                nc.sync.dma_start(ctx_d[b:b + 1, :], crow[:])

    # CoreSim can't model the raw inserted wait ops; skip the split there.
    return _split_multi_waits(nc) if split_waits else nc


# Instruction classes whose waits live outside the 64B engine encoding.
_WAIT_SPLIT_SKIP = {"InstEventSemaphore", "InstCollectiveCompute"}


def _split_multi_waits(nc):
    """The 64-byte ISA encoding has exactly ONE semaphore-wait slot per
    instruction; this walrus build refuses instructions carrying more.  Tile's
    sem-assignment can attach several, so split the extras into standalone
    EventSemaphore (wait-only) instructions on the same engine, inserted
    immediately before the over-subscribed instruction."""
    wid = 0
    for f in nc.m.functions:
        for blk in f.blocks:
            il = blk.instructions
            i = 0
            while i < len(il):
                inst = il[i]
                si = getattr(inst, "sync_info", None)
                if (si is not None and len(si.on_wait) > 1
                        and type(inst).__name__ not in _WAIT_SPLIT_SKIP):
                    for w in si.on_wait[:-1]:
                        ws = mybir.InstEventSemaphore(name=f"I-wsplit-{wid}")
                        wid += 1
                        ws.engine = inst.engine
                        ws.sync_info = mybir.SyncInfo(on_wait=[w], on_update=[])
                        il.insert(i, ws)
                        i += 1
                    inst.sync_info = mybir.SyncInfo(on_wait=si.on_wait[-1:],
                                                    on_update=si.on_update)
                i += 1
    return nc


_CACHE = {}


def kernel(encoder_out, decoder_hidden, W_enc, b_enc, W_dec, b_dec, w_full,
           b_full=None, **_ignored):
    encoder_out = np.ascontiguousarray(encoder_out, dtype=np.float32)
    decoder_hidden = np.ascontiguousarray(decoder_hidden, dtype=np.float32)
    shared = {
        "w_enc": np.ascontiguousarray(W_enc, dtype=np.float32),
        "b_enc": np.ascontiguousarray(b_enc, dtype=np.float32).reshape(1, A),
        "w_dec": np.ascontiguousarray(W_dec, dtype=np.float32),
        "b_dec": np.ascontiguousarray(b_dec, dtype=np.float32).reshape(1, A),
        "w_full": np.ascontiguousarray(w_full, dtype=np.float32).reshape(1, A),
    }
    if "nc" not in _CACHE:
        _CACHE["nc"] = build()
    nc = _CACHE["nc"]

    in_maps = []
    for c in range(N_CORES):
        sl = slice(c * BC, (c + 1) * BC)
        in_maps.append({
            "enc": encoder_out[sl],
            "dec": decoder_hidden[sl],
            **shared,
        })
    res = run_bass_kernel_spmd(nc, in_maps, list(range(N_CORES)))
    context = np.concatenate([r["context"] for r in res.results], axis=0)
    alpha = np.concatenate([r["alpha"] for r in res.results], axis=0)
    return context, alpha


# revision 30
# speedup vs baseline: 79.6365x; 59.9559x over previous
"""Bahdanau attention (context + alpha) on Trainium2, 8-core data-parallel.

Math (per batch b):
  att1[p,a]  = sum_e enc[b,p,e] * W_enc[e,a]
  att2[a]    = sum_d dec[b,d] * W_dec[d,a] + b_dec[a]
  z[p,a]     = relu(att1[p,a] + att2[a] + b_enc[a])
  att[p]     = sum_a z[p,a] * w_full[a]          (+ b_full, dropped: softmax-shift-invariant)
  alpha[p]   = softmax_p(att)
  context[e] = sum_p alpha[p] * enc[b,p,e]

Sharding: batch dim split over 8 cores (32 batches each); small weights replicated.

On-chip layout: the att1 matmul contracts over e, so encoder tiles are needed with
e on the partition axis (encT).  Natural-layout tiles (p on partitions) are loaded
with fully contiguous DMA and transposed on the PE (identity matmul).  The att1
result is produced transposed (a on partitions, pixels of a batch PAIR side by
side on the free axis: N=392) so relu-bias (per-a) is a per-partition activation
bias and the w_full projection is one K=128 matmul per a-chunk.  Softmax runs on
the (1, 392) score row without max-subtraction (scores are O(sigma)~1, exp-safe).
Context reuses the natural-layout tiles: lhsT = transposed exp-row, accumulate
over the two p-chunks, scaled by 1/sum(exp) on PSUM->SBUF copy-out.
"""

import os
from contextlib import ExitStack

import numpy as np

import concourse.bass as bass
import concourse.mybir as mybir
import concourse.tile as tile
from concourse.bass_utils import run_bass_kernel_spmd
from concourse.masks import make_identity

F32 = mybir.dt.float32
AF = mybir.ActivationFunctionType

N_CORES = 8
B, P, E, A, D = 256, 196, 2048, 512, 512
BC = B // N_CORES            # 32 batches per core
NPAIR = BC // 2              # 16 batch pairs per core
P0 = 128                     # first p-chunk rows
P1 = P - P0                  # 68
ECH = E // 128               # 16 e-chunks
ACH = A // 128               # 4 a-chunks
DCH = D // 128               # 4 d-chunks
W2 = 2 * P                   # 392: paired free width

# Matmul dtype for the att1/score matmuls: float32 (exact, 4 cyc/row) or
# float32r (reduced-precision single-pass, 1 cyc/row at N>=256).  fp32r
# operands must be produced by ops that round to fp32r (walrus birverifier
# rule), so operand tiles are allocated in MM_DT and filled by compute ops,
# never straight from DMA.  The context matmul keeps plain fp32: its rhs is
# the DMA-loaded natural-layout encoder tile.
MM_DT = getattr(mybir.dt, os.environ.get("KERNEL_MM_DT", "float32r"))


def build(split_waits=True):
    nc = bass.Bass(
        trn_type="TRN2",
        target_bir_lowering=False,
        debug=False,
        num_devices=N_CORES,
    )

    enc_d = nc.dram_tensor("enc", [BC, P, E], F32, kind="ExternalInput").ap()
    dec_d = nc.dram_tensor("dec", [BC, D], F32, kind="ExternalInput").ap()
    wenc_d = nc.dram_tensor("w_enc", [E, A], F32, kind="ExternalInput").ap()
    benc_d = nc.dram_tensor("b_enc", [1, A], F32, kind="ExternalInput").ap()
    wdec_d = nc.dram_tensor("w_dec", [D, A], F32, kind="ExternalInput").ap()
    bdec_d = nc.dram_tensor("b_dec", [1, A], F32, kind="ExternalInput").ap()
    wful_d = nc.dram_tensor("w_full", [1, A], F32, kind="ExternalInput").ap()
    ctx_d = nc.dram_tensor("context", [BC, E], F32, kind="ExternalOutput").ap()
    alp_d = nc.dram_tensor("alpha", [BC, P], F32, kind="ExternalOutput").ap()

    with tile.TileContext(nc) as tc, ExitStack() as ctx:
        const = ctx.enter_context(tc.tile_pool(name="const", bufs=1))

        ident = const.tile([128, 128], F32)
        make_identity(nc, ident[:])
        # rounded identity: fp32r transposes run 1.5 cyc/row vs 2.0 for fp32
        ident_r = const.tile([128, 128], MM_DT)
        nc.vector.tensor_copy(ident_r[:], ident[:])

        # --- replicated weights ------------------------------------------
        wenc_r = const.tile([128, ECH * A], MM_DT)  # [ep, (ec, a)], rounded
        wful_t = const.tile([128, ACH], MM_DT)  # w_full as [a_in_chunk, ac]
        beb_t = const.tile([128, ACH], F32)     # b_enc+b_dec likewise
        att2p = const.tile([128, ACH * BC], F32)  # [ap, (ac, b)]: att2+biases

        with tc.tile_pool(name="setup_sb", bufs=1) as stage, \
                tc.tile_pool(name="setup_ps", bufs=2, space="PSUM") as sps:
            wenc_sb = stage.tile([128, ECH * A], F32)  # [ep, (ec, a)]
            for ec in range(ECH):
                nc.sync.dma_start(
                    wenc_sb[:, ec * A:(ec + 1) * A],
                    wenc_d[ec * 128:(ec + 1) * 128, :],
                )
                # round fp32 -> fp32r (per chunk: keeps DVE waits single-sem)
                nc.vector.tensor_copy(wenc_r[:, ec * A:(ec + 1) * A],
                                      wenc_sb[:, ec * A:(ec + 1) * A])
            wdec_sb = stage.tile([128, DCH * A], F32)  # [dp, (dc, a)]
            nc.sync.dma_start(
                wdec_sb[:].rearrange("p (c a) -> p c a", c=DCH),
                wdec_d.rearrange("(c p) a -> p c a", p=128),
            )
            benc_r = stage.tile([1, A], F32)
            nc.sync.dma_start(benc_r[:], benc_d)
            bdec_r = stage.tile([1, A], F32)
            nc.sync.dma_start(bdec_r[:], bdec_d)
            wful_r = stage.tile([1, A], F32)
            nc.sync.dma_start(wful_r[:], wful_d)
            dec_nat = stage.tile([BC, D], F32)
            nc.sync.dma_start(dec_nat[:], dec_d)

            # bias_eb = b_enc + b_dec (both added to att1 pre-relu).  Stage
            # bdec through a DVE copy so the add waits on one semaphore only
            # (DVE TensorTensor has a single sync-wait slot).
            bdec_c = stage.tile([1, A], F32)
            nc.vector.tensor_copy(bdec_c[:], bdec_r[:])
            beb_r = stage.tile([1, A], F32)
            nc.vector.tensor_add(beb_r[:], benc_r[:], bdec_c[:])
            dect_sb = stage.tile([128, DCH * BC], F32)  # decT: [dp, (dc, b)]

            for c in range(ACH):
                t = sps.tile([128, 1], F32, tag="vec")
                nc.tensor.transpose(t[:], wful_r[:, c * 128:(c + 1) * 128],
                                    ident[0:1, 0:1])
                nc.vector.tensor_copy(wful_t[:, c:c + 1], t[:])
                t2 = sps.tile([128, 1], F32, tag="vec")
                nc.tensor.transpose(t2[:], beb_r[:, c * 128:(c + 1) * 128],
                                    ident[0:1, 0:1])
                nc.vector.tensor_copy(beb_t[:, c:c + 1], t2[:])
            for dc in range(DCH):
                t = sps.tile([128, BC], F32, tag="dec")
                nc.tensor.transpose(t[:], dec_nat[:, dc * 128:(dc + 1) * 128],
                                    ident[0:BC, 0:BC])
                nc.vector.tensor_copy(dect_sb[:, dc * BC:(dc + 1) * BC], t[:])
            # att2p[:, ac*BC + b] = (dec @ W_dec)[b, ac*128:...] + b_enc + b_dec
            for ac in range(ACH):
                t = sps.tile([128, BC], F32, tag="att2")
                for dc in range(DCH):
                    nc.tensor.matmul(
                        t[:],
                        wdec_sb[:, dc * A + ac * 128: dc * A + (ac + 1) * 128],
                        dect_sb[:, dc * BC:(dc + 1) * BC],
                        start=(dc == 0),
                        stop=(dc == DCH - 1),
                    )
                nc.vector.tensor_scalar_add(att2p[:, ac * BC:(ac + 1) * BC],
                                            t[:], beb_t[:, ac:ac + 1])

        # --- main pools ---------------------------------------------------
        stgA = ctx.enter_context(tc.tile_pool(name="stgA", bufs=2))
        stgB = ctx.enter_context(tc.tile_pool(name="stgB", bufs=2))
        natA = ctx.enter_context(tc.tile_pool(name="natA", bufs=4))
        natB = ctx.enter_context(tc.tile_pool(name="natB", bufs=4))
        encT = ctx.enter_context(tc.tile_pool(name="encT", bufs=2))
        relu_p = ctx.enter_context(tc.tile_pool(name="relu", bufs=2))
        sm_p = ctx.enter_context(tc.tile_pool(name="sm", bufs=2))
        ctxrow_p = ctx.enter_context(tc.tile_pool(name="ctxrow", bufs=2))
        aT_sb_p = ctx.enter_context(tc.tile_pool(name="aTsb", bufs=2))

        eT_ps_p = ctx.enter_context(tc.tile_pool(name="eTps", bufs=2, space="PSUM"))
        z_ps_p = ctx.enter_context(tc.tile_pool(name="zps", bufs=2, space="PSUM"))
        att_ps_p = ctx.enter_context(tc.tile_pool(name="attps", bufs=1, space="PSUM"))
        aT_ps_p = ctx.enter_context(tc.tile_pool(name="aTps", bufs=1, space="PSUM"))
        ctx_ps_p = ctx.enter_context(tc.tile_pool(name="ctxps", bufs=2, space="PSUM"))

        for i in range(NPAIR):
            b0, b1 = 2 * i, 2 * i + 1

            # DMA lands fp32 in a short-lived staging tile; a round-copy to
            # MM_DT produces the tile every on-chip consumer reads (the
            # birverifier requires fp32r matmul operands to come from a
            # rounding instruction, and DMA cannot round).  Copies alternate
            # DVE/ACT to balance engine load.
            nat = []
            for j, b in enumerate((b0, b1)):
                sa = stgA.tile([P0, E], F32, tag="stgA")
                nc.sync.dma_start(sa[:], enc_d[b, 0:P0, :])
                a_t = natA.tile([P0, E], MM_DT, tag="natA")
                nc.gpsimd.tensor_copy(a_t[:], sa[:])  # round on idle Pool
                sb = stgB.tile([P1, E], F32, tag="stgB")
                nc.sync.dma_start(sb[:], enc_d[b, P0:P, :])
                b_t = natB.tile([P1, E], MM_DT, tag="natB")
                nc.gpsimd.tensor_copy(b_t[:], sb[:])
                nat.append((a_t, b_t))

            # encT: [e_in_chunk, (ec, pair-cols)]; cols = b0 p0..p195, b1 p0..p195
            eT = encT.tile([128, ECH * W2], MM_DT, tag="encT")
            for ec in range(ECH):
                ps = eT_ps_p.tile([128, W2], MM_DT, tag="eT")
                sl = ec * 128
                nc.tensor.transpose(ps[:, 0:P0], nat[0][0][:, sl:sl + 128],
                                    ident_r[:])
                nc.tensor.transpose(ps[:, P0:P], nat[0][1][:, sl:sl + 128],
                                    ident_r[0:P1, 0:P1])
                nc.tensor.transpose(ps[:, P:P + P0], nat[1][0][:, sl:sl + 128],
                                    ident_r[:])
                nc.tensor.transpose(ps[:, P + P0:W2], nat[1][1][:, sl:sl + 128],
                                    ident_r[0:P1, 0:P1])
                nc.vector.tensor_copy(eT[:, ec * W2:(ec + 1) * W2], ps[:])

            # att scores for the pair: (1, 392) accumulated over a-chunks
            att_ps = att_ps_p.tile([1, W2], F32, tag="att")
            for ac in range(ACH):
                z = z_ps_p.tile([128, W2], F32, tag="z")
                for ec in range(ECH):
                    nc.tensor.matmul(
                        z[:],
                        wenc_r[:, ec * A + ac * 128: ec * A + (ac + 1) * 128],
                        eT[:, ec * W2:(ec + 1) * W2],
                        start=(ec == 0),
                        stop=(ec == ECH - 1),
                    )
                # r = relu(z + att2[b]) on DVE, rounding to fp32r for score mm
                r = relu_p.tile([128, W2], MM_DT, tag="relu")
                nc.vector.tensor_scalar(
                    r[:, 0:P], z[:, 0:P],
                    att2p[:, ac * BC + b0: ac * BC + b0 + 1], 0.0,
                    op0=mybir.AluOpType.add, op1=mybir.AluOpType.max)
                nc.vector.tensor_scalar(
                    r[:, P:W2], z[:, P:W2],
                    att2p[:, ac * BC + b1: ac * BC + b1 + 1], 0.0,
                    op0=mybir.AluOpType.add, op1=mybir.AluOpType.max)
                nc.tensor.matmul(att_ps[:], wful_t[:, ac:ac + 1], r[:],
                                 start=(ac == 0), stop=(ac == ACH - 1))

            # softmax over each 196-half (no max-subtraction; scores are O(1))
            exp_sb = sm_p.tile([1, W2], F32, tag="exp")
            s_sb = sm_p.tile([1, 2], F32, tag="s")
            rec = sm_p.tile([1, 2], F32, tag="rec")
            nc.scalar.activation(exp_sb[:, 0:P], att_ps[:, 0:P], AF.Exp,
                                 accum_out=s_sb[:, 0:1])
            nc.scalar.activation(exp_sb[:, P:W2], att_ps[:, P:W2], AF.Exp,
                                 accum_out=s_sb[:, 1:2])
            nc.vector.reciprocal(rec[:], s_sb[:])
            alpha_sb = sm_p.tile([1, W2], F32, tag="alpha")
            nc.vector.tensor_scalar_mul(alpha_sb[:, 0:P], exp_sb[:, 0:P],
                                        rec[:, 0:1])
            nc.vector.tensor_scalar_mul(alpha_sb[:, P:W2], exp_sb[:, P:W2],
                                        rec[:, 1:2])
            nc.sync.dma_start(alp_d[b0:b0 + 1, :], alpha_sb[0:1, 0:P])
            nc.sync.dma_start(alp_d[b1:b1 + 1, :], alpha_sb[0:1, P:W2])

            # transpose normalized alpha row -> column vectors for context
            aT_ps = aT_ps_p.tile([128, 4], F32, tag="aT")
            nc.tensor.transpose(aT_ps[:, 0:1], alpha_sb[:, 0:P0], ident[0:1, 0:1])
            nc.tensor.transpose(aT_ps[:, 1:2], alpha_sb[:, P:P + P0],
                                ident[0:1, 0:1])
            nc.tensor.transpose(aT_ps[0:P1, 2:3], alpha_sb[:, P0:P],
                                ident[0:1, 0:1])
            nc.tensor.transpose(aT_ps[0:P1, 3:4], alpha_sb[:, P + P0:W2],
                                ident[0:1, 0:1])
            aT = aT_sb_p.tile([128, 4], MM_DT, tag="aTsb")
            nc.vector.tensor_copy(aT[:, 0:2], aT_ps[:, 0:2])
            nc.vector.tensor_copy(aT[0:P1, 2:4], aT_ps[0:P1, 2:4])

            # context[b] = (sum_p exp[p] * enc[p, :]) / sum_exp
            for j, b in enumerate((b0, b1)):
                crow = ctxrow_p.tile([1, E], F32, tag="ctxrow")
                for n4 in range(4):
                    cps = ctx_ps_p.tile([1, 512], F32, tag="ctx")
                    nc.tensor.matmul(cps[:], aT[:, j:j + 1],
                                     nat[j][0][:, n4 * 512:(n4 + 1) * 512],
                                     start=True, stop=False)
                    nc.tensor.matmul(cps[:], aT[0:P1, 2 + j:3 + j],
                                     nat[j][1][:, n4 * 512:(n4 + 1) * 512],
                                     start=False, stop=True)
                    nc.scalar.activation(crow[:, n4 * 512:(n4 + 1) * 512], cps[:],
                                         AF.Copy)
                nc.sync.dma_start(ctx_d[b:b + 1, :], crow[:])

    # CoreSim can't model the raw inserted wait ops; skip the split there.
    return _split_multi_waits(nc) if split_waits else nc


# Instruction classes whose waits live outside the 64B engine encoding.
_WAIT_SPLIT_SKIP = {"InstEventSemaphore", "InstCollectiveCompute"}


def _split_multi_waits(nc):
    """The 64-byte ISA encoding has exactly ONE semaphore-wait slot per
    instruction; this walrus build refuses instructions carrying more.  Tile's
    sem-assignment can attach several, so split the extras into standalone
    EventSemaphore (wait-only) instructions on the same engine, inserted
    immediately before the over-subscribed instruction."""
    wid = 0
    for f in nc.m.functions:
        for blk in f.blocks:
            il = blk.instructions
            i = 0
            while i < len(il):
                inst = il[i]
                si = getattr(inst, "sync_info", None)
                if (si is not None and len(si.on_wait) > 1
                        and type(inst).__name__ not in _WAIT_SPLIT_SKIP):
                    for w in si.on_wait[:-1]:
                        ws = mybir.InstEventSemaphore(name=f"I-wsplit-{wid}")
                        wid += 1
                        ws.engine = inst.engine
                        ws.sync_info = mybir.SyncInfo(on_wait=[w], on_update=[])
                        il.insert(i, ws)
                        i += 1
                    inst.sync_info = mybir.SyncInfo(on_wait=si.on_wait[-1:],
                                                    on_update=si.on_update)
                i += 1
    return nc


_CACHE = {}


def kernel(encoder_out, decoder_hidden, W_enc, b_enc, W_dec, b_dec, w_full,
           b_full=None, **_ignored):
    encoder_out = np.ascontiguousarray(encoder_out, dtype=np.float32)
    decoder_hidden = np.ascontiguousarray(decoder_hidden, dtype=np.float32)
    shared = {
        "w_enc": np.ascontiguousarray(W_enc, dtype=np.float32),
        "b_enc": np.ascontiguousarray(b_enc, dtype=np.float32).reshape(1, A),
        "w_dec": np.ascontiguousarray(W_dec, dtype=np.float32),
        "b_dec": np.ascontiguousarray(b_dec, dtype=np.float32).reshape(1, A),
        "w_full": np.ascontiguousarray(w_full, dtype=np.float32).reshape(1, A),
    }
    # The axon trace path needs an NTFF hook module absent from this env;
    # make sure run_bass_kernel_spmd never takes it.
    os.environ["BASS_NEVER_TRACE"] = "1"
    if "nc" not in _CACHE:
        _CACHE["nc"] = build()
    nc = _CACHE["nc"]

    in_maps = []
    for c in range(N_CORES):
        sl = slice(c * BC, (c + 1) * BC)
        in_maps.append({
            "enc": encoder_out[sl],
            "dec": decoder_hidden[sl],
            **shared,
        })
    res = run_bass_kernel_spmd(nc, in_maps, list(range(N_CORES)))
    context = np.concatenate([r["context"] for r in res.results], axis=0)
    alpha = np.concatenate([r["alpha"] for r in res.results], axis=0)
    return context, alpha


# revision 32
# speedup vs baseline: 117.3259x; 1.4733x over previous
"""Bahdanau attention (context + alpha) on Trainium2, 8-core data-parallel.

Math (per batch b):
  att1[p,a]  = sum_e enc[b,p,e] * W_enc[e,a]
  att2[a]    = sum_d dec[b,d] * W_dec[d,a] + b_dec[a]
  z[p,a]     = relu(att1[p,a] + att2[a] + b_enc[a])
  att[p]     = sum_a z[p,a] * w_full[a]          (+ b_full, dropped: softmax-shift-invariant)
  alpha[p]   = softmax_p(att)
  context[e] = sum_p alpha[p] * enc[b,p,e]

Sharding: batch dim split over 8 cores (32 batches each); small weights replicated.

On-chip layout: the att1 matmul contracts over e, so encoder tiles are needed with
e on the partition axis (encT).  Natural-layout tiles (p on partitions) are loaded
with fully contiguous DMA and transposed on the PE (identity matmul).  The att1
result is produced transposed (a on partitions, pixels of a batch PAIR side by
side on the free axis: N=392) so relu-bias (per-a) is a per-partition activation
bias and the w_full projection is one K=128 matmul per a-chunk.  Softmax runs on
the (1, 392) score row without max-subtraction (scores are O(sigma)~1, exp-safe).
Context reuses the natural-layout tiles: lhsT = transposed exp-row, accumulate
over the two p-chunks, scaled by 1/sum(exp) on PSUM->SBUF copy-out.
"""

import os
from contextlib import ExitStack

import numpy as np

import concourse.bass as bass
import concourse.mybir as mybir
import concourse.tile as tile
from concourse.bass_utils import run_bass_kernel_spmd
from concourse.masks import make_identity

F32 = mybir.dt.float32
AF = mybir.ActivationFunctionType

N_CORES = 8
B, P, E, A, D = 256, 196, 2048, 512, 512
BC = B // N_CORES            # 32 batches per core
NPAIR = BC // 2              # 16 batch pairs per core
P0 = 128                     # first p-chunk rows
P1 = P - P0                  # 68
ECH = E // 128               # 16 e-chunks
ACH = A // 128               # 4 a-chunks
DCH = D // 128               # 4 d-chunks
W2 = 2 * P                   # 392: paired free width

# Matmul dtype for the att1/score matmuls: float32 (exact, 4 cyc/row) or
# float32r (reduced-precision single-pass, 1 cyc/row at N>=256).  fp32r
# operands must be produced by ops that round to fp32r (walrus birverifier
# rule), so operand tiles are allocated in MM_DT and filled by compute ops,
# never straight from DMA.  The context matmul keeps plain fp32: its rhs is
# the DMA-loaded natural-layout encoder tile.
MM_DT = getattr(mybir.dt, os.environ.get("KERNEL_MM_DT", "float32r"))


def build(split_waits=True):
    nc = bass.Bass(
        trn_type="TRN2",
        target_bir_lowering=False,
        debug=False,
        num_devices=N_CORES,
    )

    enc_d = nc.dram_tensor("enc", [BC, P, E], F32, kind="ExternalInput").ap()
    dec_d = nc.dram_tensor("dec", [BC, D], F32, kind="ExternalInput").ap()
    wenc_d = nc.dram_tensor("w_enc", [E, A], F32, kind="ExternalInput").ap()
    benc_d = nc.dram_tensor("b_enc", [1, A], F32, kind="ExternalInput").ap()
    wdec_d = nc.dram_tensor("w_dec", [D, A], F32, kind="ExternalInput").ap()
    bdec_d = nc.dram_tensor("b_dec", [1, A], F32, kind="ExternalInput").ap()
    wful_d = nc.dram_tensor("w_full", [1, A], F32, kind="ExternalInput").ap()
    ctx_d = nc.dram_tensor("context", [BC, E], F32, kind="ExternalOutput").ap()
    alp_d = nc.dram_tensor("alpha", [BC, P], F32, kind="ExternalOutput").ap()

    with tile.TileContext(nc) as tc, ExitStack() as ctx:
        const = ctx.enter_context(tc.tile_pool(name="const", bufs=1))

        ident = const.tile([128, 128], F32)
        make_identity(nc, ident[:])
        # rounded identity: fp32r transposes run 1.5 cyc/row vs 2.0 for fp32
        ident_r = const.tile([128, 128], MM_DT)
        nc.vector.tensor_copy(ident_r[:], ident[:])

        # --- replicated weights ------------------------------------------
        wenc_r = const.tile([128, ECH * A], MM_DT)  # [ep, (ec, a)], rounded
        wful_t = const.tile([128, ACH], MM_DT)  # w_full as [a_in_chunk, ac]
        beb_t = const.tile([128, ACH], F32)     # b_enc+b_dec likewise
        att2p = const.tile([128, ACH * BC], F32)  # [ap, (ac, b)]: att2+biases

        with tc.tile_pool(name="setup_sb", bufs=1) as stage, \
                tc.tile_pool(name="setup_ps", bufs=2, space="PSUM") as sps:
            wenc_sb = stage.tile([128, ECH * A], F32)  # [ep, (ec, a)]
            for ec in range(ECH):
                nc.sync.dma_start(
                    wenc_sb[:, ec * A:(ec + 1) * A],
                    wenc_d[ec * 128:(ec + 1) * 128, :],
                )
                # round fp32 -> fp32r (per chunk: keeps DVE waits single-sem)
                nc.vector.tensor_copy(wenc_r[:, ec * A:(ec + 1) * A],
                                      wenc_sb[:, ec * A:(ec + 1) * A])
            wdec_sb = stage.tile([128, DCH * A], F32)  # [dp, (dc, a)]
            nc.sync.dma_start(
                wdec_sb[:].rearrange("p (c a) -> p c a", c=DCH),
                wdec_d.rearrange("(c p) a -> p c a", p=128),
            )
            benc_r = stage.tile([1, A], F32)
            nc.sync.dma_start(benc_r[:], benc_d)
            bdec_r = stage.tile([1, A], F32)
            nc.sync.dma_start(bdec_r[:], bdec_d)
            wful_r = stage.tile([1, A], F32)
            nc.sync.dma_start(wful_r[:], wful_d)
            dec_nat = stage.tile([BC, D], F32)
            nc.sync.dma_start(dec_nat[:], dec_d)

            # bias_eb = b_enc + b_dec (both added to att1 pre-relu).  Stage
            # bdec through a DVE copy so the add waits on one semaphore only
            # (DVE TensorTensor has a single sync-wait slot).
            bdec_c = stage.tile([1, A], F32)
            nc.vector.tensor_copy(bdec_c[:], bdec_r[:])
            beb_r = stage.tile([1, A], F32)
            nc.vector.tensor_add(beb_r[:], benc_r[:], bdec_c[:])
            dect_sb = stage.tile([128, DCH * BC], F32)  # decT: [dp, (dc, b)]

            for c in range(ACH):
                t = sps.tile([128, 1], F32, tag="vec")
                nc.tensor.transpose(t[:], wful_r[:, c * 128:(c + 1) * 128],
                                    ident[0:1, 0:1])
                nc.vector.tensor_copy(wful_t[:, c:c + 1], t[:])
                t2 = sps.tile([128, 1], F32, tag="vec")
                nc.tensor.transpose(t2[:], beb_r[:, c * 128:(c + 1) * 128],
                                    ident[0:1, 0:1])
                nc.vector.tensor_copy(beb_t[:, c:c + 1], t2[:])
            for dc in range(DCH):
                t = sps.tile([128, BC], F32, tag="dec")
                nc.tensor.transpose(t[:], dec_nat[:, dc * 128:(dc + 1) * 128],
                                    ident[0:BC, 0:BC])
                nc.vector.tensor_copy(dect_sb[:, dc * BC:(dc + 1) * BC], t[:])
            # att2p[:, ac*BC + b] = (dec @ W_dec)[b, ac*128:...] + b_enc + b_dec
            for ac in range(ACH):
                t = sps.tile([128, BC], F32, tag="att2")
                for dc in range(DCH):
                    nc.tensor.matmul(
                        t[:],
                        wdec_sb[:, dc * A + ac * 128: dc * A + (ac + 1) * 128],
                        dect_sb[:, dc * BC:(dc + 1) * BC],
                        start=(dc == 0),
                        stop=(dc == DCH - 1),
                    )
                nc.vector.tensor_scalar_add(att2p[:, ac * BC:(ac + 1) * BC],
                                            t[:], beb_t[:, ac:ac + 1])

        # --- main pools ---------------------------------------------------
        stgA = ctx.enter_context(tc.tile_pool(name="stgA", bufs=2))
        stgB = ctx.enter_context(tc.tile_pool(name="stgB", bufs=2))
        natA = ctx.enter_context(tc.tile_pool(name="natA", bufs=4))
        natB = ctx.enter_context(tc.tile_pool(name="natB", bufs=4))
        encT = ctx.enter_context(tc.tile_pool(name="encT", bufs=2))
        relu_p = ctx.enter_context(tc.tile_pool(name="relu", bufs=2))
        sm_p = ctx.enter_context(tc.tile_pool(name="sm", bufs=2))
        ctxrow_p = ctx.enter_context(tc.tile_pool(name="ctxrow", bufs=2))
        aT_sb_p = ctx.enter_context(tc.tile_pool(name="aTsb", bufs=2))

        eT_ps_p = ctx.enter_context(tc.tile_pool(name="eTps", bufs=2, space="PSUM"))
        z_ps_p = ctx.enter_context(tc.tile_pool(name="zps", bufs=2, space="PSUM"))
        att_ps_p = ctx.enter_context(tc.tile_pool(name="attps", bufs=1, space="PSUM"))
        aT_ps_p = ctx.enter_context(tc.tile_pool(name="aTps", bufs=1, space="PSUM"))
        ctx_ps_p = ctx.enter_context(tc.tile_pool(name="ctxps", bufs=2, space="PSUM"))

        for i in range(NPAIR):
            b0, b1 = 2 * i, 2 * i + 1

            # DMA lands fp32 in a short-lived staging tile; a round-copy to
            # MM_DT produces the tile every on-chip consumer reads (the
            # birverifier requires fp32r matmul operands to come from a
            # rounding instruction, and DMA cannot round).  Copies alternate
            # DVE/ACT to balance engine load.
            nat = []
            for j, b in enumerate((b0, b1)):
                sa = stgA.tile([P0, E], F32, tag="stgA")
                nc.sync.dma_start(sa[:], enc_d[b, 0:P0, :])
                a_t = natA.tile([P0, E], MM_DT, tag="natA")
                nc.gpsimd.tensor_copy(a_t[:], sa[:])  # round on idle Pool
                sb = stgB.tile([P1, E], F32, tag="stgB")
                nc.sync.dma_start(sb[:], enc_d[b, P0:P, :])
                b_t = natB.tile([P1, E], MM_DT, tag="natB")
                nc.gpsimd.tensor_copy(b_t[:], sb[:])
                nat.append((a_t, b_t))

            # encT: [e_in_chunk, (ec, pair-cols)]; cols = b0 p0..p195, b1 p0..p195
            eT = encT.tile([128, ECH * W2], MM_DT, tag="encT")
            for ec in range(ECH):
                ps = eT_ps_p.tile([128, W2], MM_DT, tag="eT")
                sl = ec * 128
                nc.tensor.transpose(ps[:, 0:P0], nat[0][0][:, sl:sl + 128],
                                    ident_r[:])
                nc.tensor.transpose(ps[:, P0:P], nat[0][1][:, sl:sl + 128],
                                    ident_r[0:P1, 0:P1])
                nc.tensor.transpose(ps[:, P:P + P0], nat[1][0][:, sl:sl + 128],
                                    ident_r[:])
                nc.tensor.transpose(ps[:, P + P0:W2], nat[1][1][:, sl:sl + 128],
                                    ident_r[0:P1, 0:P1])
                nc.vector.tensor_copy(eT[:, ec * W2:(ec + 1) * W2], ps[:])

            # att scores for the pair: (1, 392) accumulated over a-chunks
            att_ps = att_ps_p.tile([1, W2], F32, tag="att")
            for ac in range(ACH):
                z = z_ps_p.tile([128, W2], F32, tag="z")
                for ec in range(ECH):
                    nc.tensor.matmul(
                        z[:],
                        wenc_r[:, ec * A + ac * 128: ec * A + (ac + 1) * 128],
                        eT[:, ec * W2:(ec + 1) * W2],
                        start=(ec == 0),
                        stop=(ec == ECH - 1),
                    )
                # r = relu(z + att2[b]) on DVE, rounding to fp32r for score mm
                r = relu_p.tile([128, W2], MM_DT, tag="relu")
                nc.vector.tensor_scalar(
                    r[:, 0:P], z[:, 0:P],
                    att2p[:, ac * BC + b0: ac * BC + b0 + 1], 0.0,
                    op0=mybir.AluOpType.add, op1=mybir.AluOpType.max)
                nc.vector.tensor_scalar(
                    r[:, P:W2], z[:, P:W2],
                    att2p[:, ac * BC + b1: ac * BC + b1 + 1], 0.0,
                    op0=mybir.AluOpType.add, op1=mybir.AluOpType.max)
                nc.tensor.matmul(att_ps[:], wful_t[:, ac:ac + 1], r[:],
                                 start=(ac == 0), stop=(ac == ACH - 1))

            # softmax over each 196-half (no max-subtraction; scores are O(1))
            exp_sb = sm_p.tile([1, W2], F32, tag="exp")
            s_sb = sm_p.tile([1, 2], F32, tag="s")
            rec = sm_p.tile([1, 2], F32, tag="rec")
            nc.scalar.activation(exp_sb[:, 0:P], att_ps[:, 0:P], AF.Exp,
                                 accum_out=s_sb[:, 0:1])
            nc.scalar.activation(exp_sb[:, P:W2], att_ps[:, P:W2], AF.Exp,
                                 accum_out=s_sb[:, 1:2])
            nc.vector.reciprocal(rec[:], s_sb[:])
            alpha_sb = sm_p.tile([1, W2], F32, tag="alpha")
            nc.vector.tensor_scalar_mul(alpha_sb[:, 0:P], exp_sb[:, 0:P],
                                        rec[:, 0:1])
            nc.vector.tensor_scalar_mul(alpha_sb[:, P:W2], exp_sb[:, P:W2],
                                        rec[:, 1:2])
            nc.sync.dma_start(alp_d[b0:b0 + 1, :], alpha_sb[0:1, 0:P])
            nc.sync.dma_start(alp_d[b1:b1 + 1, :], alpha_sb[0:1, P:W2])

            # transpose normalized alpha row -> column vectors for context
            aT_ps = aT_ps_p.tile([128, 4], F32, tag="aT")
            nc.tensor.transpose(aT_ps[:, 0:1], alpha_sb[:, 0:P0], ident[0:1, 0:1])
            nc.tensor.transpose(aT_ps[:, 1:2], alpha_sb[:, P:P + P0],
                                ident[0:1, 0:1])
            nc.tensor.transpose(aT_ps[0:P1, 2:3], alpha_sb[:, P0:P],
                                ident[0:1, 0:1])
            nc.tensor.transpose(aT_ps[0:P1, 3:4], alpha_sb[:, P + P0:W2],
                                ident[0:1, 0:1])
            aT = aT_sb_p.tile([128, 4], MM_DT, tag="aTsb")
            nc.vector.tensor_copy(aT[:, 0:2], aT_ps[:, 0:2])
            nc.vector.tensor_copy(aT[0:P1, 2:4], aT_ps[0:P1, 2:4])

            # context[b] = (sum_p alpha[p] * enc[p, :])
            for j, b in enumerate((b0, b1)):
                crow = ctxrow_p.tile([1, E], F32, tag="ctxrow")
                for n4 in range(4):
                    cps = ctx_ps_p.tile([1, 512], F32, tag="ctx")
                    nc.tensor.matmul(cps[:], aT[:, j:j + 1],
                                     nat[j][0][:, n4 * 512:(n4 + 1) * 512],
                                     start=True, stop=False)
                    nc.tensor.matmul(cps[:], aT[0:P1, 2 + j:3 + j],
                                     nat[j][1][:, n4 * 512:(n4 + 1) * 512],
                                     start=False, stop=True)
                    nc.scalar.activation(crow[:, n4 * 512:(n4 + 1) * 512], cps[:],
                                         AF.Copy)
                nc.sync.dma_start(ctx_d[b:b + 1, :], crow[:])

    # CoreSim can't model the raw inserted wait ops; skip the split there.
    return _split_multi_waits(nc) if split_waits else nc


# Instruction classes whose waits live outside the 64B engine encoding.
_WAIT_SPLIT_SKIP = {"InstEventSemaphore", "InstCollectiveCompute"}


def _split_multi_waits(nc):
    """The 64-byte ISA encoding has exactly ONE semaphore-wait slot per
    instruction; this walrus build refuses instructions carrying more.  Tile's
    sem-assignment can attach several, so split the extras into standalone
    EventSemaphore (wait-only) instructions on the same engine, inserted
    immediately before the over-subscribed instruction."""
    wid = 0
    for f in nc.m.functions:
        for blk in f.blocks:
            il = blk.instructions
            i = 0
            while i < len(il):
                inst = il[i]
                si = getattr(inst, "sync_info", None)
                if (si is not None and len(si.on_wait) > 1
                        and type(inst).__name__ not in _WAIT_SPLIT_SKIP):
                    for w in si.on_wait[:-1]:
                        ws = mybir.InstEventSemaphore(name=f"I-wsplit-{wid}")
                        wid += 1
                        ws.engine = inst.engine
                        ws.sync_info = mybir.SyncInfo(on_wait=[w], on_update=[])
                        il.insert(i, ws)
                        i += 1
                    inst.sync_info = mybir.SyncInfo(on_wait=si.on_wait[-1:],
                                                    on_update=si.on_update)
                i += 1
    return nc


_CACHE = {}


def kernel(encoder_out, decoder_hidden, W_enc, b_enc, W_dec, b_dec, w_full,
           b_full=None, **_ignored):
    encoder_out = np.ascontiguousarray(encoder_out, dtype=np.float32)
    decoder_hidden = np.ascontiguousarray(decoder_hidden, dtype=np.float32)
    shared = {
        "w_enc": np.ascontiguousarray(W_enc, dtype=np.float32),
        "b_enc": np.ascontiguousarray(b_enc, dtype=np.float32).reshape(1, A),
        "w_dec": np.ascontiguousarray(W_dec, dtype=np.float32),
        "b_dec": np.ascontiguousarray(b_dec, dtype=np.float32).reshape(1, A),
        "w_full": np.ascontiguousarray(w_full, dtype=np.float32).reshape(1, A),
    }
    # The axon trace path needs an NTFF hook module absent from this env;
    # make sure run_bass_kernel_spmd never takes it.
    os.environ["BASS_NEVER_TRACE"] = "1"
    if "nc" not in _CACHE:
        _CACHE["nc"] = build()
    nc = _CACHE["nc"]

    in_maps = []
    for c in range(N_CORES):
        sl = slice(c * BC, (c + 1) * BC)
        in_maps.append({
            "enc": encoder_out[sl],
            "dec": decoder_hidden[sl],
            **shared,
        })
    res = run_bass_kernel_spmd(nc, in_maps, list(range(N_CORES)))
    context = np.concatenate([r["context"] for r in res.results], axis=0)
    alpha = np.concatenate([r["alpha"] for r in res.results], axis=0)
    return context, alpha


# revision 34
# speedup vs baseline: 123.8493x; 1.0556x over previous
"""Bahdanau attention (context + alpha) on Trainium2, 8-core data-parallel.

Math (per batch b):
  att1[p,a]  = sum_e enc[b,p,e] * W_enc[e,a]
  att2[a]    = sum_d dec[b,d] * W_dec[d,a] + b_dec[a]
  z[p,a]     = relu(att1[p,a] + att2[a] + b_enc[a])
  att[p]     = sum_a z[p,a] * w_full[a]          (+ b_full, dropped: softmax-shift-invariant)
  alpha[p]   = softmax_p(att)
  context[e] = sum_p alpha[p] * enc[b,p,e]

Sharding: batch dim split over 8 cores (32 batches each); small weights replicated.

On-chip layout: the att1 matmul contracts over e, so encoder tiles are needed with
e on the partition axis (encT).  Natural-layout tiles (p on partitions) are loaded
with fully contiguous DMA and transposed on the PE (identity matmul).  The att1
result is produced transposed (a on partitions, pixels of a batch PAIR side by
side on the free axis: N=392) so relu-bias (per-a) is a per-partition activation
bias and the w_full projection is one K=128 matmul per a-chunk.  Softmax runs on
the (1, 392) score row without max-subtraction (scores are O(sigma)~1, exp-safe).
Context reuses the natural-layout tiles: lhsT = transposed exp-row, accumulate
over the two p-chunks, scaled by 1/sum(exp) on PSUM->SBUF copy-out.
"""

import os
from contextlib import ExitStack

import numpy as np

import concourse.bass as bass
import concourse.mybir as mybir
import concourse.tile as tile
from concourse.bass_utils import run_bass_kernel_spmd
from concourse.masks import make_identity

F32 = mybir.dt.float32
AF = mybir.ActivationFunctionType

N_CORES = 8
B, P, E, A, D = 256, 196, 2048, 512, 512
BC = B // N_CORES            # 32 batches per core
NPAIR = BC // 2              # 16 batch pairs per core
P0 = 128                     # first p-chunk rows
P1 = P - P0                  # 68
ECH = E // 128               # 16 e-chunks
ACH = A // 128               # 4 a-chunks
DCH = D // 128               # 4 d-chunks
W2 = 2 * P                   # 392: paired free width

# Matmul dtype for the att1/score matmuls: float32 (exact, 4 cyc/row) or
# float32r (reduced-precision single-pass, 1 cyc/row at N>=256).  fp32r
# operands must be produced by ops that round to fp32r (walrus birverifier
# rule), so operand tiles are allocated in MM_DT and filled by compute ops,
# never straight from DMA.  The context matmul keeps plain fp32: its rhs is
# the DMA-loaded natural-layout encoder tile.
MM_DT = getattr(mybir.dt, os.environ.get("KERNEL_MM_DT", "float32r"))


def build(split_waits=True):
    nc = bass.Bass(
        trn_type="TRN2",
        target_bir_lowering=False,
        debug=False,
        num_devices=N_CORES,
    )

    enc_d = nc.dram_tensor("enc", [BC, P, E], F32, kind="ExternalInput").ap()
    dec_d = nc.dram_tensor("dec", [BC, D], F32, kind="ExternalInput").ap()
    wenc_d = nc.dram_tensor("w_enc", [E, A], F32, kind="ExternalInput").ap()
    benc_d = nc.dram_tensor("b_enc", [1, A], F32, kind="ExternalInput").ap()
    wdec_d = nc.dram_tensor("w_dec", [D, A], F32, kind="ExternalInput").ap()
    bdec_d = nc.dram_tensor("b_dec", [1, A], F32, kind="ExternalInput").ap()
    wful_d = nc.dram_tensor("w_full", [1, A], F32, kind="ExternalInput").ap()
    ctx_d = nc.dram_tensor("context", [BC, E], F32, kind="ExternalOutput").ap()
    alp_d = nc.dram_tensor("alpha", [BC, P], F32, kind="ExternalOutput").ap()

    with tile.TileContext(nc) as tc, ExitStack() as ctx:
        const = ctx.enter_context(tc.tile_pool(name="const", bufs=1))

        ident = const.tile([128, 128], F32)
        make_identity(nc, ident[:])
        # rounded identity: fp32r transposes run 1.5 cyc/row vs 2.0 for fp32
        ident_r = const.tile([128, 128], MM_DT)
        nc.vector.tensor_copy(ident_r[:], ident[:])

        # --- replicated weights ------------------------------------------
        wenc_r = const.tile([128, ECH * A], MM_DT)  # [ep, (ec, a)], rounded
        wful_t = const.tile([128, ACH], MM_DT)  # w_full as [a_in_chunk, ac]
        beb_t = const.tile([128, ACH], F32)     # b_enc+b_dec likewise
        att2p = const.tile([128, ACH * BC], F32)  # [ap, (ac, b)]: att2+biases

        with tc.tile_pool(name="setup_sb", bufs=1) as stage, \
                tc.tile_pool(name="setup_ps", bufs=2, space="PSUM") as sps:
            wenc_sb = stage.tile([128, ECH * A], F32)  # [ep, (ec, a)]
            for ec in range(ECH):
                nc.sync.dma_start(
                    wenc_sb[:, ec * A:(ec + 1) * A],
                    wenc_d[ec * 128:(ec + 1) * 128, :],
                )
                # round fp32 -> fp32r (per chunk: keeps DVE waits single-sem)
                nc.vector.tensor_copy(wenc_r[:, ec * A:(ec + 1) * A],
                                      wenc_sb[:, ec * A:(ec + 1) * A])
            wdec_sb = stage.tile([128, DCH * A], F32)  # [dp, (dc, a)]
            nc.sync.dma_start(
                wdec_sb[:].rearrange("p (c a) -> p c a", c=DCH),
                wdec_d.rearrange("(c p) a -> p c a", p=128),
            )
            benc_r = stage.tile([1, A], F32)
            nc.sync.dma_start(benc_r[:], benc_d)
            bdec_r = stage.tile([1, A], F32)
            nc.sync.dma_start(bdec_r[:], bdec_d)
            wful_r = stage.tile([1, A], F32)
            nc.sync.dma_start(wful_r[:], wful_d)
            dec_nat = stage.tile([BC, D], F32)
            nc.sync.dma_start(dec_nat[:], dec_d)

            # bias_eb = b_enc + b_dec (both added to att1 pre-relu).  Stage
            # bdec through a DVE copy so the add waits on one semaphore only
            # (DVE TensorTensor has a single sync-wait slot).
            bdec_c = stage.tile([1, A], F32)
            nc.vector.tensor_copy(bdec_c[:], bdec_r[:])
            beb_r = stage.tile([1, A], F32)
            nc.vector.tensor_add(beb_r[:], benc_r[:], bdec_c[:])
            dect_sb = stage.tile([128, DCH * BC], F32)  # decT: [dp, (dc, b)]

            for c in range(ACH):
                t = sps.tile([128, 1], F32, tag="vec")
                nc.tensor.transpose(t[:], wful_r[:, c * 128:(c + 1) * 128],
                                    ident[0:1, 0:1])
                nc.vector.tensor_copy(wful_t[:, c:c + 1], t[:])
                t2 = sps.tile([128, 1], F32, tag="vec")
                nc.tensor.transpose(t2[:], beb_r[:, c * 128:(c + 1) * 128],
                                    ident[0:1, 0:1])
                nc.vector.tensor_copy(beb_t[:, c:c + 1], t2[:])
            for dc in range(DCH):
                t = sps.tile([128, BC], F32, tag="dec")
                nc.tensor.transpose(t[:], dec_nat[:, dc * 128:(dc + 1) * 128],
                                    ident[0:BC, 0:BC])
                nc.vector.tensor_copy(dect_sb[:, dc * BC:(dc + 1) * BC], t[:])
            # att2p[:, ac*BC + b] = (dec @ W_dec)[b, ac*128:...] + b_enc + b_dec
            for ac in range(ACH):
                t = sps.tile([128, BC], F32, tag="att2")
                for dc in range(DCH):
                    nc.tensor.matmul(
                        t[:],
                        wdec_sb[:, dc * A + ac * 128: dc * A + (ac + 1) * 128],
                        dect_sb[:, dc * BC:(dc + 1) * BC],
                        start=(dc == 0),
                        stop=(dc == DCH - 1),
                    )
                nc.vector.tensor_scalar_add(att2p[:, ac * BC:(ac + 1) * BC],
                                            t[:], beb_t[:, ac:ac + 1])

        # --- main pools ---------------------------------------------------
        stgA = ctx.enter_context(tc.tile_pool(name="stgA", bufs=2))
        stgB = ctx.enter_context(tc.tile_pool(name="stgB", bufs=2))
        natA = ctx.enter_context(tc.tile_pool(name="natA", bufs=4))
        natB = ctx.enter_context(tc.tile_pool(name="natB", bufs=4))
        encT = ctx.enter_context(tc.tile_pool(name="encT", bufs=2))
        relu_p = ctx.enter_context(tc.tile_pool(name="relu", bufs=2))
        sm_p = ctx.enter_context(tc.tile_pool(name="sm", bufs=2))
        ctxrow_p = ctx.enter_context(tc.tile_pool(name="ctxrow", bufs=2))
        aT_sb_p = ctx.enter_context(tc.tile_pool(name="aTsb", bufs=2))

        eT_ps_p = ctx.enter_context(tc.tile_pool(name="eTps", bufs=2, space="PSUM"))
        z_ps_p = ctx.enter_context(tc.tile_pool(name="zps", bufs=2, space="PSUM"))
        att_ps_p = ctx.enter_context(tc.tile_pool(name="attps", bufs=1, space="PSUM"))
        aT_ps_p = ctx.enter_context(tc.tile_pool(name="aTps", bufs=1, space="PSUM"))
        ctx_ps_p = ctx.enter_context(tc.tile_pool(name="ctxps", bufs=2, space="PSUM"))

        for i in range(NPAIR):
            b0, b1 = 2 * i, 2 * i + 1

            # DMA lands fp32 in a short-lived staging tile; a round-copy to
            # MM_DT produces the tile every on-chip consumer reads (the
            # birverifier requires fp32r matmul operands to come from a
            # rounding instruction, and DMA cannot round).  Copies alternate
            # DVE/ACT to balance engine load.
            nat = []
            for j, b in enumerate((b0, b1)):
                sa = stgA.tile([P0, E], F32, tag="stgA")
                nc.sync.dma_start(sa[:], enc_d[b, 0:P0, :])
                a_t = natA.tile([P0, E], MM_DT, tag="natA")
                nc.gpsimd.tensor_copy(a_t[:], sa[:])  # round on idle Pool
                sb = stgB.tile([P1, E], F32, tag="stgB")
                nc.sync.dma_start(sb[:], enc_d[b, P0:P, :])
                b_t = natB.tile([P1, E], MM_DT, tag="natB")
                nc.gpsimd.tensor_copy(b_t[:], sb[:])
                nat.append((a_t, b_t))

            # encT: [e_in_chunk, (ec, pair-cols)]; cols = b0 p0..p195, b1 p0..p195
            # Per-chunk tiles (not one big tile): an att1 matmul then only
            # depends on its own chunk's PSUM->SBUF copy, so PE isn't gated
            # on the whole DVE copy queue.
            eTs = []
            for ec in range(ECH):
                ps = eT_ps_p.tile([128, W2], MM_DT, tag="eT")
                sl = ec * 128
                nc.tensor.transpose(ps[:, 0:P0], nat[0][0][:, sl:sl + 128],
                                    ident_r[:])
                nc.tensor.transpose(ps[:, P0:P], nat[0][1][:, sl:sl + 128],
                                    ident_r[0:P1, 0:P1])
                nc.tensor.transpose(ps[:, P:P + P0], nat[1][0][:, sl:sl + 128],
                                    ident_r[:])
                nc.tensor.transpose(ps[:, P + P0:W2], nat[1][1][:, sl:sl + 128],
                                    ident_r[0:P1, 0:P1])
                et = encT.tile([128, W2], MM_DT, tag=f"e{ec}")
                nc.vector.tensor_copy(et[:], ps[:])
                eTs.append(et)

            # att scores for the pair: (1, 392) accumulated over a-chunks
            att_ps = att_ps_p.tile([1, W2], F32, tag="att")
            for ac in range(ACH):
                z = z_ps_p.tile([128, W2], F32, tag="z")
                for ec in range(ECH):
                    nc.tensor.matmul(
                        z[:],
                        wenc_r[:, ec * A + ac * 128: ec * A + (ac + 1) * 128],
                        eTs[ec][:],
                        start=(ec == 0),
                        stop=(ec == ECH - 1),
                    )
                # r = relu(z + att2[b]) on DVE, rounding to fp32r for score mm
                r = relu_p.tile([128, W2], MM_DT, tag="relu")
                nc.vector.tensor_scalar(
                    r[:, 0:P], z[:, 0:P],
                    att2p[:, ac * BC + b0: ac * BC + b0 + 1], 0.0,
                    op0=mybir.AluOpType.add, op1=mybir.AluOpType.max)
                nc.vector.tensor_scalar(
                    r[:, P:W2], z[:, P:W2],
                    att2p[:, ac * BC + b1: ac * BC + b1 + 1], 0.0,
                    op0=mybir.AluOpType.add, op1=mybir.AluOpType.max)
                nc.tensor.matmul(att_ps[:], wful_t[:, ac:ac + 1], r[:],
                                 start=(ac == 0), stop=(ac == ACH - 1))

            # softmax over each 196-half (no max-subtraction; scores are O(1))
            exp_sb = sm_p.tile([1, W2], F32, tag="exp")
            s_sb = sm_p.tile([1, 2], F32, tag="s")
            rec = sm_p.tile([1, 2], F32, tag="rec")
            nc.scalar.activation(exp_sb[:, 0:P], att_ps[:, 0:P], AF.Exp,
                                 accum_out=s_sb[:, 0:1])
            nc.scalar.activation(exp_sb[:, P:W2], att_ps[:, P:W2], AF.Exp,
                                 accum_out=s_sb[:, 1:2])
            nc.vector.reciprocal(rec[:], s_sb[:])
            alpha_sb = sm_p.tile([1, W2], F32, tag="alpha")
            nc.vector.tensor_scalar_mul(alpha_sb[:, 0:P], exp_sb[:, 0:P],
                                        rec[:, 0:1])
            nc.vector.tensor_scalar_mul(alpha_sb[:, P:W2], exp_sb[:, P:W2],
                                        rec[:, 1:2])
            nc.sync.dma_start(alp_d[b0:b0 + 1, :], alpha_sb[0:1, 0:P])
            nc.sync.dma_start(alp_d[b1:b1 + 1, :], alpha_sb[0:1, P:W2])

            # transpose normalized alpha row -> column vectors for context
            aT_ps = aT_ps_p.tile([128, 4], F32, tag="aT")
            nc.tensor.transpose(aT_ps[:, 0:1], alpha_sb[:, 0:P0], ident[0:1, 0:1])
            nc.tensor.transpose(aT_ps[:, 1:2], alpha_sb[:, P:P + P0],
                                ident[0:1, 0:1])
            nc.tensor.transpose(aT_ps[0:P1, 2:3], alpha_sb[:, P0:P],
                                ident[0:1, 0:1])
            nc.tensor.transpose(aT_ps[0:P1, 3:4], alpha_sb[:, P + P0:W2],
                                ident[0:1, 0:1])
            aT = aT_sb_p.tile([128, 4], MM_DT, tag="aTsb")
            nc.vector.tensor_copy(aT[:, 0:2], aT_ps[:, 0:2])
            nc.vector.tensor_copy(aT[0:P1, 2:4], aT_ps[0:P1, 2:4])

            # context[b] = (sum_p alpha[p] * enc[p, :])
            for j, b in enumerate((b0, b1)):
                crow = ctxrow_p.tile([1, E], F32, tag="ctxrow")
                for n4 in range(4):
                    cps = ctx_ps_p.tile([1, 512], F32, tag="ctx")
                    nc.tensor.matmul(cps[:], aT[:, j:j + 1],
                                     nat[j][0][:, n4 * 512:(n4 + 1) * 512],
                                     start=True, stop=False)
                    nc.tensor.matmul(cps[:], aT[0:P1, 2 + j:3 + j],
                                     nat[j][1][:, n4 * 512:(n4 + 1) * 512],
                                     start=False, stop=True)
                    nc.scalar.activation(crow[:, n4 * 512:(n4 + 1) * 512], cps[:],
                                         AF.Copy)
                nc.sync.dma_start(ctx_d[b:b + 1, :], crow[:])

    # CoreSim can't model the raw inserted wait ops; skip the split there.
    return _split_multi_waits(nc) if split_waits else nc


# Instruction classes whose waits live outside the 64B engine encoding.
_WAIT_SPLIT_SKIP = {"InstEventSemaphore", "InstCollectiveCompute"}


def _split_multi_waits(nc):
    """The 64-byte ISA encoding has exactly ONE semaphore-wait slot per
    instruction; this walrus build refuses instructions carrying more.  Tile's
    sem-assignment can attach several, so split the extras into standalone
    EventSemaphore (wait-only) instructions on the same engine, inserted
    immediately before the over-subscribed instruction."""
    wid = 0
    for f in nc.m.functions:
        for blk in f.blocks:
            il = blk.instructions
            i = 0
            while i < len(il):
                inst = il[i]
                si = getattr(inst, "sync_info", None)
                if (si is not None and len(si.on_wait) > 1
                        and type(inst).__name__ not in _WAIT_SPLIT_SKIP):
                    for w in si.on_wait[:-1]:
                        ws = mybir.InstEventSemaphore(name=f"I-wsplit-{wid}")
                        wid += 1
                        ws.engine = inst.engine
                        ws.sync_info = mybir.SyncInfo(on_wait=[w], on_update=[])
                        il.insert(i, ws)
                        i += 1
                    inst.sync_info = mybir.SyncInfo(on_wait=si.on_wait[-1:],
                                                    on_update=si.on_update)
                i += 1
    return nc


_CACHE = {}


def kernel(encoder_out, decoder_hidden, W_enc, b_enc, W_dec, b_dec, w_full,
           b_full=None, **_ignored):
    encoder_out = np.ascontiguousarray(encoder_out, dtype=np.float32)
    decoder_hidden = np.ascontiguousarray(decoder_hidden, dtype=np.float32)
    shared = {
        "w_enc": np.ascontiguousarray(W_enc, dtype=np.float32),
        "b_enc": np.ascontiguousarray(b_enc, dtype=np.float32).reshape(1, A),
        "w_dec": np.ascontiguousarray(W_dec, dtype=np.float32),
        "b_dec": np.ascontiguousarray(b_dec, dtype=np.float32).reshape(1, A),
        "w_full": np.ascontiguousarray(w_full, dtype=np.float32).reshape(1, A),
    }
    # The axon trace path needs an NTFF hook module absent from this env;
    # make sure run_bass_kernel_spmd never takes it.
    os.environ["BASS_NEVER_TRACE"] = "1"
    if "nc" not in _CACHE:
        _CACHE["nc"] = build()
    nc = _CACHE["nc"]

    in_maps = []
    for c in range(N_CORES):
        sl = slice(c * BC, (c + 1) * BC)
        in_maps.append({
            "enc": encoder_out[sl],
            "dec": decoder_hidden[sl],
            **shared,
        })
    res = run_bass_kernel_spmd(nc, in_maps, list(range(N_CORES)))
    context = np.concatenate([r["context"] for r in res.results], axis=0)
    alpha = np.concatenate([r["alpha"] for r in res.results], axis=0)
    return context, alpha
